# revision 1
# baseline (speedup 1.0000x reference)
"""CrossModalAttention Trainium2 kernel (8 NeuronCores, tensor-parallel heads).

Strategy:
  - Head-parallel: 16 heads / 8 cores = 2 heads per core for both attention
    passes. Each core computes its 128-channel slice of Q/K/V for both
    modalities (QKV weights column-sliced per core, activations replicated).
  - All matmul operands are pre-transposed on the host so every device matmul
    contracts over the partition dim with no on-device layout fixups:
      QT/KT = W_slice-chunks @ X^T  -> [128ch, 8192tok] channel-major.
  - Attention computed as ST = scores^T [k, q]; exp(ST) feeds the A@V matmul
    directly as the moving operand. A ones-column appended to V yields the
    softmax denominators inside the same PSUM accumulation.
  - No max-subtraction in softmax: logits here are ~N(0,1); exp is safe.
  - fused = f_ta + f_tb written token-major per batch; AllToAll redistributes
    to token-parallel layout; LayerNorm (gamma/beta folded into W_proj on
    host) + projection run distributed over tokens.
  - Matmuls run in float32r (1 cyc/row at N=512 vs 4 for fp32); everything
    around them (softmax normalization, LayerNorm, PSUM) stays fp32.
"""

import numpy as np

import concourse.bacc as bacc
import concourse.mybir as mybir
import concourse.tile as tile
from concourse.bass_utils import run_bass_kernel_spmd

NC = 8            # cores
B = 4             # batch
N = 2048          # seq len
T = B * N         # total tokens = 8192
D = 1024          # model dim
H = 16            # heads
HPC = H // NC     # heads per core = 2
HD = D // H       # head dim = 64
S = D // NC       # channel slice per core = 128
SCALE = HD ** -0.5
KC = 16           # k chunks of 128 per batch
QC = 4            # q chunks of 512 per batch
DK = 8            # contraction chunks of 128 over D
EPS = 1e-5

F32 = mybir.dt.float32
F32R = mybir.dt.float32r
MM = F32R  # matmul operand dtype


def _build_nc(single=False):
    """single=True: 1-core variant for TimelineSim (collective replaced by a
    local DMA copy of the same buffers) — timing analysis only."""
    nc = bacc.Bacc("TRN2", target_bir_lowering=False, debug=False,
                   num_devices=(1 if single else NC))

    # ---- I/O ----
    xt_a = nc.dram_tensor("xt_a", [128, DK, T], MM, kind="ExternalInput")
    xt_b = nc.dram_tensor("xt_b", [128, DK, T], MM, kind="ExternalInput")
    wnames = ["wq_a", "wk_a", "wv_a", "wq_b", "wk_b", "wv_b"]
    w_in = {m: nc.dram_tensor(m, [128, DK, S], MM, kind="ExternalInput") for m in wnames}
    b_in = {m: nc.dram_tensor("b" + m[1:], [S, 1], F32, kind="ExternalInput") for m in wnames}
    wpt = nc.dram_tensor("wpt", [128, DK, D], MM, kind="ExternalInput")
    beff = nc.dram_tensor("beff", [1, D], MM, kind="ExternalInput")
    ident_r_in = nc.dram_tensor("ident_r", [128, 128], MM, kind="ExternalInput")
    ident_f_in = nc.dram_tensor("ident_f", [128, 128], F32, kind="ExternalInput")
    onescol_in = nc.dram_tensor("onescol", [128, KC, HPC, 1], MM, kind="ExternalInput")
    onesrow_in = nc.dram_tensor("onesrow", [1, 128], MM, kind="ExternalInput")
    out = nc.dram_tensor("out", [T // NC, D], F32, kind="ExternalOutput")

    with tile.TileContext(nc) as tc:
        with (
            tc.tile_pool(name="const", bufs=1) as constp,
            tc.tile_pool(name="sb", bufs=1) as sb,
            tc.tile_pool(name="ps", bufs=1, space="PSUM") as ps,
            tc.tile_pool(name="dram", bufs=1, space="DRAM") as dram,
        ):
            # ---- constants ----
            ident_r = constp.tile([128, 128], MM)
            nc.sync.dma_start(ident_r[:], ident_r_in[:])
            ident_f = constp.tile([128, 128], F32)
            nc.sync.dma_start(ident_f[:], ident_f_in[:])
            onesrow = constp.tile([1, 128], MM)
            nc.sync.dma_start(onesrow[:], onesrow_in[:])
            beff_sb = constp.tile([1, D], MM)
            nc.sync.dma_start(beff_sb[:], beff[:])

            # ---- weights resident in SBUF ----
            wsb = {}
            bsb = {}
            for m in wnames:
                w = sb.tile([128, DK, S], MM, name=f"w_{m}", tag=f"w_{m}")
                nc.sync.dma_start(w[:], w_in[m][:])
                wsb[m] = w
                bt = sb.tile([S, 1], F32, name=f"b_{m}", tag=f"b_{m}")
                nc.sync.dma_start(bt[:], b_in[m][:])
                bsb[m] = bt

            # ---- internal DRAM: channel-major projections [128, T] ----
            proj_dram = {}
            for nm in ["qta", "ktb", "vtb", "qtb", "kta", "vta"]:
                proj_dram[nm] = dram.tile([128, T], MM, name=f"d_{nm}", tag=f"d_{nm}")
            fused_d = [dram.tile([N, S], F32, name=f"fused{b}", tag=f"fused{b}") for b in range(B)]
            a2a_d = [dram.tile([N, S], F32, name=f"a2a{b}", tag=f"a2a{b}") for b in range(B)]

            # ================= Phase 1: QKV projections =================
            # dst = (W_slice @ X^T) + bias : [128 ch, T] channel-major
            plan = [
                (xt_a, [("wq_a", "qta"), ("wk_a", "kta"), ("wv_a", "vta")]),
                (xt_b, [("wk_b", "ktb"), ("wv_b", "vtb"), ("wq_b", "qtb")]),
            ]
            # Phase 1 is emitted batch-interleaved with phase 2 so the
            # scheduler round-robins PE between projections and attention
            # and ACT (exp) starts early.
            TW = 512

            def emit_proj_chunk(tch, sbuf_dst=None):
                for src, projs in plan:
                    xs = sb.tile([128, DK, TW], MM, name="xs", tag="big16", bufs=2)
                    nc.sync.dma_start(xs[:], src[:, :, tch * TW:(tch + 1) * TW])
                    for m, dst in projs:
                        pp = ps.tile([128, TW], F32, name="pp", tag="pp", bufs=1)
                        for k in range(DK):
                            nc.tensor.matmul(
                                pp[:], wsb[m][:, k, :], xs[:, k, :],
                                start=(k == 0), stop=(k == DK - 1),
                            )
                        if sbuf_dst is not None:
                            # write straight into the attention SBUF tile
                            # (skips the DRAM round trip on the critical path)
                            t, c0 = sbuf_dst[dst]
                            nc.vector.tensor_scalar_add(
                                t[:, c0 + tch * TW:c0 + (tch + 1) * TW],
                                pp[:], bsb[m][:, 0:1])
                        else:
                            ob = sb.tile([128, TW], MM, name="ob", tag="ob", bufs=3)
                            nc.vector.tensor_scalar_add(ob[:], pp[:], bsb[m][:, 0:1])
                            nc.sync.dma_start(
                                proj_dram[dst][:, tch * TW:(tch + 1) * TW], ob[:]
                            )

            # ================= Phase 2: attention =================
            # pass 0: f_ta = attn(Qa, Kb, Vb); pass 1: f_tb = attn(Qb, Ka, Va)
            osb_all = {b: {} for b in range(B)}

            def emit_attn_loads(b, pas):
                qt, kt, vt = (("qta", "ktb", "vtb") if pas == 0
                              else ("qtb", "kta", "vta"))
                qsb = sb.tile([128, N], MM, name="qsb", tag="qsb", bufs=2)
                nc.sync.dma_start(qsb[:], proj_dram[qt][:, b * N:(b + 1) * N])
                ksb = sb.tile([128, N], MM, name="ksb", tag="ksb", bufs=2)
                nc.sync.dma_start(ksb[:], proj_dram[kt][:, b * N:(b + 1) * N])
                vsb = sb.tile([128, N], MM, name="vsb", tag="vsb", bufs=2)
                nc.sync.dma_start(vsb[:], proj_dram[vt][:, b * N:(b + 1) * N])
                return qsb, ksb, vsb

            def emit_attn_pass(b, pas, loads):
                osb = osb_all[b]
                qsb, ksb, vsb = loads

                # V^T [128ch, N] -> token-major V_aug tiles [128k, 65]
                # (col 64 = 1.0 for softmax denominators)
                vaug = sb.tile([128, KC, HPC, 65], MM, name="vaug", tag="vaug", bufs=2)
                nc.sync.dma_start(vaug[:, :, :, 64:65], onescol_in[:])
                for kc in range(KC):
                    tp = ps.tile([128, 128], MM, name="tp", tag="misc", bufs=1)
                    nc.tensor.transpose(tp[:], vsb[:, kc * 128:(kc + 1) * 128], ident_r[:])
                    for h in range(HPC):
                        nc.vector.tensor_copy(vaug[:, kc, h, 0:64], tp[:, h * HD:(h + 1) * HD])

                for h in range(HPC):
                    hsl = slice(h * HD, (h + 1) * HD)
                    for qc in range(QC):
                        qsl = slice(qc * 512, (qc + 1) * 512)
                        av = ps.tile([65, 512], F32, name="av", tag="av", bufs=2)
                        for kc2 in range(KC // 2):
                            # two score chunks into one 2-bank PSUM tile,
                            # one wide exp (halves ACT per-op overhead)
                            sp = ps.tile([128, 1024], F32, name="sp", tag="sp", bufs=2)
                            ex = sb.tile([128, 1024], MM, name="ex", tag="t512", bufs=4)
                            for j in range(2):
                                kc = kc2 * 2 + j
                                nc.tensor.matmul(
                                    sp[:, j * 512:(j + 1) * 512],
                                    ksb[hsl, kc * 128:(kc + 1) * 128],
                                    qsb[hsl, qsl],
                                    start=True, stop=True,
                                )
                            nc.scalar.activation(
                                ex[:], sp[:], mybir.ActivationFunctionType.Exp,
                                scale=SCALE,
                            )
                            for j in range(2):
                                kc = kc2 * 2 + j
                                nc.tensor.matmul(
                                    av[:], vaug[:, kc, h, :],
                                    ex[:, j * 512:(j + 1) * 512],
                                    start=(kc == 0), stop=(kc == KC - 1),
                                )
                        # drain PSUM with one copy so the av slot frees
                        # fast, then normalize from SBUF
                        avs = sb.tile([65, 512], F32, name="avs", tag="avs", bufs=3)
                        nc.vector.tensor_copy(avs[:], av[:])
                        rec = sb.tile([1, 512], F32, name="rec", tag="rec", bufs=2)
                        nc.vector.reciprocal(rec[:], avs[64:65, :])
                        rbs = sb.tile([64, 512], F32, name="rbs", tag="rbs", bufs=2)
                        nc.gpsimd.partition_broadcast(rbs[:], rec[:])
                        if pas == 0:
                            o = sb.tile([64, 512], F32, name="o",
                                        tag=f"osb{h}{qc}", bufs=1)
                            nc.vector.tensor_mul(o[:], avs[0:64, :], rbs[:])
                            osb[(h, qc)] = o
                        else:
                            fs = sb.tile([64, 512], F32, name="fs", tag="fs", bufs=2)
                            nc.vector.tensor_mul(fs[:], avs[0:64, :], rbs[:])
                            nc.vector.tensor_add(fs[:], fs[:], osb[(h, qc)][:])
                            # token-major transpose + store fused slice
                            for qi in range(4):
                                ftp = ps.tile([128, 64], F32, name="ftp",
                                              tag="misc", bufs=1)
                                nc.tensor.transpose(
                                    ftp[:], fs[:, qi * 128:(qi + 1) * 128],
                                    ident_f[0:64, 0:64],
                                )
                                fts = sb.tile([128, 64], F32, name="fts",
                                              tag="fts", bufs=3)
                                nc.vector.tensor_copy(fts[:], ftp[:])
                                r0 = qc * 512 + qi * 128
                                nc.sync.dma_start(
                                    fused_d[b][r0:r0 + 128, h * HD:(h + 1) * HD],
                                    fts[:],
                                )

            def emit_a2a(b):
                if single:
                    nc.sync.dma_start(a2a_d[b][:], fused_d[b][:])
                else:
                    nc.gpsimd.collective_compute(
                        "AllToAll", mybir.AluOpType.bypass,
                        replica_groups=[list(range(NC))],
                        ins=[fused_d[b].opt()], outs=[a2a_d[b].opt()],
                    )

            # interleave: proj chunks for batch b, then that batch's Q/K/V
            # loads (ahead of the next proj burst in the DMA queues), with
            # proj bursts emitted after the attention pass they overlap
            q0 = sb.tile([128, N], MM, name="q0", tag="qsb", bufs=2)
            k0 = sb.tile([128, N], MM, name="k0", tag="ksb", bufs=2)
            v0 = sb.tile([128, N], MM, name="v0", tag="vsb", bufs=2)
            q1 = sb.tile([128, N], MM, name="q1", tag="qsb", bufs=2)
            k1 = sb.tile([128, N], MM, name="k1", tag="ksb", bufs=2)
            v1 = sb.tile([128, N], MM, name="v1", tag="vsb", bufs=2)
            b0_dst = {"qta": (q0, 0), "ktb": (k0, 0), "vtb": (v0, 0),
                      "qtb": (q1, 0), "kta": (k1, 0), "vta": (v1, 0)}
            for tch in range(4):
                emit_proj_chunk(tch, sbuf_dst=b0_dst)   # batch 0 -> SBUF
            l00 = (q0, k0, v0)
            l01 = (q1, k1, v1)
            emit_attn_pass(0, 0, l00)
            for tch in range(4, 8):
                emit_proj_chunk(tch)          # batch 1 columns
            emit_attn_pass(0, 1, l01)
            emit_a2a(0)
            l10 = emit_attn_loads(1, 0)
            l11 = emit_attn_loads(1, 1)
            emit_attn_pass(1, 0, l10)
            for tch in range(8, 12):
                emit_proj_chunk(tch)          # batch 2 columns
            emit_attn_pass(1, 1, l11)
            emit_a2a(1)
            l20 = emit_attn_loads(2, 0)
            l21 = emit_attn_loads(2, 1)
            emit_attn_pass(2, 0, l20)
            emit_attn_pass(2, 1, l21)
            for tch in range(12, 16):
                emit_proj_chunk(tch)          # batch 3 columns
            emit_a2a(2)
            l30 = emit_attn_loads(3, 0)
            l31 = emit_attn_loads(3, 1)
            emit_attn_pass(3, 0, l30)
            emit_attn_pass(3, 1, l31)
            emit_a2a(3)

            # ================= Phase 3: LayerNorm + projection =================
            # a2a_d[b] rows [i*256:(i+1)*256] = channel block i of this core's
            # 256 tokens of batch b.
            TB = N // NC  # 256 tokens per core per batch
            # wpt loaded now (reuses the big16 slots freed after phase 1)
            wph = []
            for oc in range(2):
                wp = sb.tile([128, DK, 512], MM, name=f"wph{oc}", tag="big16", bufs=2)
                nc.sync.dma_start(wp[:], wpt[:, :, oc * 512:(oc + 1) * 512])
                wph.append(wp)
            for b in range(B):
                av3 = a2a_d[b][:].rearrange("(i r) c -> r i c", i=NC)  # [256, 8, 128]
                for tt in range(TB // 128):
                    x = sb.tile([128, NC, S], F32, name="x", tag="x", bufs=2)
                    nc.sync.dma_start(x[:], av3[tt * 128:(tt + 1) * 128])
                    xf = x[:].rearrange("p i c -> p (i c)")  # [128, 1024]
                    ssum = sb.tile([128, 1], F32, name="ssum", tag="stat", bufs=4)
                    nc.vector.reduce_sum(ssum[:], xf, axis=mybir.AxisListType.X)
                    mu = sb.tile([128, 1], F32, name="mu", tag="stat", bufs=4)
                    nc.vector.tensor_scalar_mul(mu[:], ssum[:], 1.0 / D)
                    xc = sb.tile([128, D], F32, name="xc", tag="xc", bufs=2)
                    nc.vector.tensor_scalar(xc[:], xf, mu[:, 0:1], None,
                                            op0=mybir.AluOpType.subtract)
                    sq = sb.tile([128, D], F32, name="sq", tag="x", bufs=2)
                    nc.vector.tensor_mul(sq[:], xc[:], xc[:])
                    vs = sb.tile([128, 1], F32, name="vs", tag="stat", bufs=4)
                    nc.vector.reduce_sum(vs[:], sq[:], axis=mybir.AxisListType.X)
                    var = sb.tile([128, 1], F32, name="var", tag="stat", bufs=4)
                    nc.vector.tensor_scalar(var[:], vs[:], 1.0 / D, EPS,
                                            op0=mybir.AluOpType.mult,
                                            op1=mybir.AluOpType.add)
                    inv = sb.tile([128, 1], F32, name="inv", tag="stat", bufs=4)
                    nc.vector.reciprocal(inv[:], var[:])
                    rstd = sb.tile([128, 1], F32, name="rstd", tag="stat", bufs=4)
                    nc.scalar.sqrt(rstd[:], inv[:])
                    xn = sb.tile([128, D], MM, name="xn", tag="xc", bufs=2)
                    nc.vector.tensor_scalar_mul(xn[:], xc[:], rstd[:, 0:1])
                    # transpose to [d, t] chunks
                    fnT = sb.tile([128, DK, 128], MM, name="fnT", tag="fnT", bufs=2)
                    for k in range(DK):
                        tp3 = ps.tile([128, 128], MM, name="tp3", tag="misc", bufs=1)
                        nc.tensor.transpose(tp3[:], xn[:, k * 128:(k + 1) * 128], ident_r[:])
                        nc.vector.tensor_copy(fnT[:, k, :], tp3[:])
                    # out tile rows
                    r0 = b * TB + tt * 128
                    for oc in range(2):
                        osl = slice(oc * 512, (oc + 1) * 512)
                        pp3 = ps.tile([128, 512], F32, name="pp3", tag="pp", bufs=1)
                        for k in range(DK):
                            nc.tensor.matmul(pp3[:], fnT[:, k, :],
                                             wph[oc][:, k, :],
                                             start=(k == 0), stop=False)
                        nc.tensor.matmul(pp3[:], onesrow[0:1, :],
                                         beff_sb[0:1, osl],
                                         start=False, stop=True)
                        ou = sb.tile([128, 512], F32, name="ou", tag="t512", bufs=4)
                        nc.vector.tensor_copy(ou[:], pp3[:])
                        nc.sync.dma_start(out[r0:r0 + 128, osl], ou[:])

    nc.compile()
    return nc


_NC_CACHE = None


def _get_nc():
    global _NC_CACHE
    if _NC_CACHE is None:
        _NC_CACHE = _build_nc()
    return _NC_CACHE


def _prep_w(w):
    """[out_rows, D] weight slice -> transposed chunked [128, DK, out] f32."""
    wt = np.ascontiguousarray(w.T)          # [D, out]
    o = wt.shape[1]
    return np.ascontiguousarray(
        wt.reshape(DK, 128, o).transpose(1, 0, 2), dtype=np.float32
    )


def _make_in_maps(inputs):
    f_a = np.asarray(inputs["f_a"], np.float32).reshape(T, D)
    f_b = np.asarray(inputs["f_b"], np.float32).reshape(T, D)
    gamma = np.asarray(inputs["ln_gamma"], np.float32)
    beta = np.asarray(inputs["ln_beta"], np.float32)
    W_proj = np.asarray(inputs["W_proj"], np.float32)
    b_proj = np.asarray(inputs["b_proj"], np.float32)

    xt_a = _prep_w(f_a)  # [128, DK, T]
    xt_b = _prep_w(f_b)

    # fold LN gamma/beta into projection: y = (ln01(x)*g+bt) @ Wp^T + bp
    #   = ln01(x) @ (Wp*g)^T + (bt @ Wp^T + bp)
    w_eff = W_proj * gamma[None, :]
    b_eff = (b_proj + beta @ W_proj.T).astype(np.float32)
    wpt = _prep_w(w_eff)  # [128, DK, D]
    ident = np.eye(128, dtype=np.float32)

    wmap = {"wq_a": "W_q_a", "wk_a": "W_k_a", "wv_a": "W_v_a",
            "wq_b": "W_q_b", "wk_b": "W_k_b", "wv_b": "W_v_b"}

    in_maps = []
    for c in range(NC):
        sl = slice(c * S, (c + 1) * S)
        m = {"xt_a": xt_a, "xt_b": xt_b, "wpt": wpt,
             "beff": b_eff.reshape(1, D),
             "ident_r": ident, "ident_f": ident,
             "onescol": np.ones((128, KC, HPC, 1), np.float32),
             "onesrow": np.ones((1, 128), np.float32)}
        for dev_name, inp_name in wmap.items():
            m[dev_name] = _prep_w(np.asarray(inputs[inp_name], np.float32)[sl, :])
            m["b" + dev_name[1:]] = np.ascontiguousarray(
                np.asarray(inputs["b" + inp_name[1:]], np.float32)[sl].reshape(S, 1)
            )
        in_maps.append(m)
    return in_maps


def _assemble(outs):
    """outs: list of per-core 'out' arrays [T//NC, D] -> [B, N, D]."""
    TB = N // NC
    full = np.empty((T, D), np.float32)
    for c in range(NC):
        oc = outs[c].reshape(B, TB, D)
        for b in range(B):
            full[b * N + c * TB: b * N + (c + 1) * TB] = oc[b]
    return full.reshape(B, N, D)


def kernel(**inputs):
    in_maps = _make_in_maps(inputs)
    nc = _get_nc()
    res = run_bass_kernel_spmd(nc, in_maps, list(range(NC)))
    return _assemble([res.results[c]["out"] for c in range(NC)])



# revision 18
# speedup vs baseline: 1.0398x; 1.0398x over previous
"""CrossModalAttention Trainium2 kernel (8 NeuronCores, tensor-parallel heads).

Strategy (v2, bf16):
  - Head-parallel: 16 heads / 8 cores = 2 heads per core for both attention
    passes. All matmul operands bf16 (1 cyc/row incl. N<256), PSUM fp32.
  - Q/K projected channel-major ([128ch, T]) and kept resident in SBUF; V
    projected *token-major* directly (x^T chunks stationary, W_v^T moving)
    into ones-augmented [128k, 65] stationary tiles -- no PE transposes and
    no DRAM round-trip for any of Q/K/V.
  - Scores computed as S^T [k, q] with the two heads emitted as row-tiled
    matmul pairs (tile_position derives from base partitions 0/64): the sim
    costs them serially but real HW runs them concurrently (2x scores).
  - exp on ACT in [128, 1024] tiles (PSUM -> SBUF bf16); softmax denominators
    come free from the 65th (ones) column of the V stationary inside the
    A@V accumulation.
  - fused = f_ta + f_tb accumulated channel-major in SBUF; per-batch bf16
    AllToAll redistributes [8 destcore, 128ch, 256tok] blocks to
    token-parallel layout.
  - LayerNorm epilogue is channel-major and transpose-free: Sigma(x) and
    Sigma(x^2) via ones-column matmuls; rstd = exp(-0.5*ln(var+eps)) (Log and
    Exp share one ACT table set, so no table thrash against attention exps);
    mean/bias are folded into the projection PSUM via one rank-2 matmul
    (wsum (x) (-mu) + beff (x) (1/rstd)), then one fused DVE scale by rstd.
  - Output stays channel-major per core ([8 occ, 128, 1024 tok]); the host
    transposes when assembling the full [B, N, D] result.
"""

import numpy as np
import ml_dtypes

import concourse.bacc as bacc
import concourse.mybir as mybir
import concourse.tile as tile
from concourse.bass_utils import run_bass_kernel_spmd

NC = 8            # cores
B = 4             # batch
N = 2048          # seq len
T = B * N         # total tokens = 8192
D = 1024          # model dim
H = 16            # heads
HPC = H // NC     # heads per core = 2
HD = D // H       # head dim = 64
S = D // NC       # channel slice per core = 128
SCALE = HD ** -0.5
KCB = 16          # k chunks of 128 per batch
KCT = T // 128    # k chunks of 128 total = 64
QC = 4            # q chunks of 512 per batch
DK = 8            # contraction chunks of 128 over D
TB = N // NC      # tokens per core per batch after A2A = 256
EPS = 1e-5

F32 = mybir.dt.float32
BF16 = mybir.dt.bfloat16
AF = mybir.ActivationFunctionType
ALU = mybir.AluOpType


def _build_nc(single=False):
    """single=True: 1-core variant for TimelineSim (collective replaced by a
    local DMA copy of the same buffers) -- timing analysis only."""
    nc = bacc.Bacc("TRN2", target_bir_lowering=False, debug=False,
                   num_devices=(1 if single else NC))

    # ---- I/O ----
    xt = {m: nc.dram_tensor(f"xt_{m}", [128, DK, T], BF16, kind="ExternalInput")
          for m in "ab"}
    wq = {m: nc.dram_tensor(f"wq_{m}", [128, DK, S], BF16, kind="ExternalInput")
          for m in "ab"}
    wk = {m: nc.dram_tensor(f"wk_{m}", [128, DK, S], BF16, kind="ExternalInput")
          for m in "ab"}
    wv = {m: nc.dram_tensor(f"wv_{m}", [128, DK, S], BF16, kind="ExternalInput")
          for m in "ab"}
    bq = {m: nc.dram_tensor(f"bq_{m}", [S, 1], F32, kind="ExternalInput")
          for m in "ab"}
    bk = {m: nc.dram_tensor(f"bk_{m}", [S, 1], F32, kind="ExternalInput")
          for m in "ab"}
    bv = {m: nc.dram_tensor(f"bv_{m}", [1, S], BF16, kind="ExternalInput")
          for m in "ab"}
    wpt_in = nc.dram_tensor("wpt", [128, DK, D], BF16, kind="ExternalInput")
    wb2_in = nc.dram_tensor("wb2", [2, DK, 128], BF16, kind="ExternalInput")
    onesrow_in = nc.dram_tensor("onesrow", [1, 128], BF16, kind="ExternalInput")
    ones128_in = nc.dram_tensor("ones128", [128, 1], BF16, kind="ExternalInput")
    out = nc.dram_tensor("out", [DK, 128, B * TB], F32, kind="ExternalOutput")

    with tile.TileContext(nc) as tc:
        with (
            tc.tile_pool(name="const", bufs=1) as constp,
            tc.tile_pool(name="sb", bufs=1) as sb,
            tc.tile_pool(name="ps", bufs=1, space="PSUM") as ps,
            tc.tile_pool(name="dram", bufs=1, space="DRAM") as dram,
        ):
            # ---- constants / weights resident in SBUF ----
            onesrow = constp.tile([1, 128], BF16)
            nc.sync.dma_start(onesrow[:], onesrow_in[:])
            ones128 = constp.tile([128, 1], BF16)
            nc.sync.dma_start(ones128[:], ones128_in[:])
            wb2 = constp.tile([2, DK, 128], BF16)
            nc.sync.dma_start(wb2[:], wb2_in[:])
            wsb = {}
            bsb = {}
            for m in "ab":
                for nm, src in (("q", wq), ("k", wk), ("v", wv)):
                    w = sb.tile([128, DK, S], BF16, name=f"w{nm}{m}", tag=f"w{nm}{m}")
                    nc.sync.dma_start(w[:], src[m][:])
                    wsb[nm + m] = w
                for nm, src in (("q", bq), ("k", bk)):
                    bt = sb.tile([S, 1], F32, name=f"b{nm}{m}", tag=f"b{nm}{m}")
                    nc.sync.dma_start(bt[:], src[m][:])
                    bsb[nm + m] = bt
                bt = sb.tile([1, S], BF16, name=f"bv{m}", tag=f"bv{m}")
                nc.sync.dma_start(bt[:], bv[m][:])
                bsb["v" + m] = bt
            # wpt tile is allocated here but its (16KB/partition) DMA is
            # emitted after batch 0's QKV chunks so it doesn't delay them
            wpt = sb.tile([128, DK, D], BF16, name="wpt", tag="wpt")
            epsc = constp.tile([1, 1], F32)
            nc.vector.memset(epsc[:], EPS)

            # ---- persistent activations ----
            qsb = {m: sb.tile([128, T], BF16, name=f"qsb{m}", tag=f"qsb{m}")
                   for m in "ab"}
            ksb = {m: sb.tile([128, T], BF16, name=f"ksb{m}", tag=f"ksb{m}")
                   for m in "ab"}
            # token-major ones-augmented V: [128 ktok, kchunk, head, 65]
            vaug = {m: sb.tile([128, KCT, HPC, 65], BF16, name=f"vaug{m}",
                               tag=f"vaug{m}") for m in "ab"}
            for m in "ab":
                nc.vector.memset(vaug[m][:, :, :, 64:65], 1.0)

            # ---- internal DRAM for the collective ----
            # [destcore*128 + channel, tok]: AllToAll splits dim0 into 8 chunks
            fused_d = [dram.tile([NC * 128, TB], BF16, name=f"fused{b}",
                                 tag=f"fused{b}") for b in range(B)]
            a2a_d = [dram.tile([NC * 128, TB], BF16, name=f"a2a{b}",
                               tag=f"a2a{b}") for b in range(B)]

            # ================= QKV projection chunk =================
            def emit_qkv_chunk(m, ch):
                sl = slice(ch * 512, (ch + 1) * 512)
                xs = sb.tile([128, DK, 512], BF16, name="xs", tag="xs", bufs=2)
                nc.sync.dma_start(xs[:], xt[m][:, :, sl])
                # Q, K channel-major: [128ch, 512tok]
                for nm, dst in (("q", qsb[m]), ("k", ksb[m])):
                    pq = ps.tile([128, 512], F32, name="pq", tag="pqkv", bufs=2)
                    for k in range(DK):
                        nc.tensor.matmul(pq[:], wsb[nm + m][:, k, :], xs[:, k, :],
                                         start=(k == 0), stop=(k == DK - 1))
                    nc.gpsimd.tensor_scalar_add(dst[:, sl], pq[:],
                                                bsb[nm + m][:, 0:1])
                # V token-major: 4 x [128tok, 128ch], bias via ones-row matmul
                pv = ps.tile([128, 4, 128], F32, name="pv", tag="pqkv", bufs=2)
                for j in range(4):
                    tsl = slice(j * 128, (j + 1) * 128)
                    for k in range(DK):
                        nc.tensor.matmul(pv[:, j, :], xs[:, k, tsl],
                                         wsb["v" + m][:, k, :],
                                         start=(k == 0), stop=False)
                    nc.tensor.matmul(pv[:, j, :], onesrow[0:1, :],
                                     bsb["v" + m][0:1, :],
                                     start=False, stop=True)
                nc.gpsimd.tensor_copy(
                    vaug[m][:, ch * 4:(ch + 1) * 4, :, 0:64],
                    pv[:].rearrange("p j (h d) -> p j h d", h=HPC))

            # ================= attention =================
            # pass 0: f_ta = attn(Qa, Kb, Vb); pass 1: f_tb = attn(Qb, Ka, Va)
            fused_sb = {}

            def emit_attn_qc(b, p, qc, qkv_feeds=()):
                qsrc, ksrc, vsrc = ((qsb["a"], ksb["b"], vaug["b"]) if p == 0
                                    else (qsb["b"], ksb["a"], vaug["a"]))
                fsb = fused_sb[b]
                qsl = slice(b * N + qc * 512, b * N + (qc + 1) * 512)
                feed_at = {3: None, 7: None, 11: None}
                for i, f in enumerate(qkv_feeds):
                    feed_at[(3, 7, 11)[i]] = f
                av = {h: ps.tile([65, 512], F32, name=f"av{h}", tag=f"av{h}",
                                 bufs=1) for h in range(HPC)}
                for kc in range(KCB):
                    ksl = slice(b * N + kc * 128, b * N + kc * 128 + 128)
                    sp = ps.tile([128, 1024], F32, name="sp", tag="sp", bufs=2)
                    for h in range(HPC):
                        hsl = slice(h * HD, (h + 1) * HD)
                        nc.tensor.matmul(sp[:, h * 512:(h + 1) * 512],
                                         ksrc[hsl, ksl], qsrc[hsl, qsl],
                                         start=True, stop=True)
                    ex = sb.tile([128, 1024], BF16, name="ex", tag="ex", bufs=4)
                    nc.scalar.activation(ex[:], sp[:], AF.Exp, scale=SCALE)
                    for h in range(HPC):
                        nc.tensor.matmul(av[h][:],
                                         vsrc[:, b * KCB + kc, h, :],
                                         ex[:, h * 512:(h + 1) * 512],
                                         start=(kc == 0), stop=(kc == KCB - 1))
                    # interleave QKV chunks of the next batch mid-stream
                    if feed_at.get(kc):
                        emit_qkv_chunk(*feed_at[kc])
                for h in range(HPC):
                    # quick one-op drain so the av PSUM bank frees fast;
                    # normalization then runs off the critical path from SBUF
                    avs = sb.tile([65, 512], F32, name="avs", tag="avs", bufs=4)
                    nc.gpsimd.tensor_copy(avs[:], av[h][:])
                    rec = sb.tile([1, 512], F32, name="rec", tag="rec", bufs=2)
                    nc.vector.reciprocal(rec[:], avs[64:65, :])
                    rbs = sb.tile([64, 512], F32, name="rbs", tag="rbs", bufs=2)
                    nc.gpsimd.partition_broadcast(rbs[:], rec[:])
                    fsl = fsb[h * HD:(h + 1) * HD, qc * 512:(qc + 1) * 512]
                    if p == 0:
                        nc.vector.tensor_mul(fsl, avs[0:64, :], rbs[:])
                    else:
                        fs = sb.tile([64, 512], F32, name="fs", tag="fs", bufs=2)
                        nc.vector.tensor_mul(fs[:], avs[0:64, :], rbs[:])
                        nc.vector.tensor_add(fsl, fsl, fs[:])
                        for dc2 in range(2):
                            dc = qc * 2 + dc2
                            r0 = dc * 128 + h * HD
                            nc.sync.dma_start(
                                fused_d[b][r0:r0 + HD, :],
                                fsb[h * HD:(h + 1) * HD,
                                    qc * 512 + dc2 * TB:qc * 512 + (dc2 + 1) * TB])

            def emit_a2a(b):
                if single:
                    nc.sync.dma_start(a2a_d[b][:], fused_d[b][:])
                else:
                    nc.gpsimd.collective_compute(
                        "AllToAll", ALU.bypass,
                        replica_groups=[list(range(NC))],
                        ins=[fused_d[b].opt()], outs=[a2a_d[b].opt()],
                    )

            # ================= LayerNorm + projection epilogue =================
            def emit_epilogue(b):
                x = sb.tile([128, DK, TB], BF16, name="xep", tag="xep", bufs=2)
                nc.sync.dma_start(
                    x[:], a2a_d[b][:].rearrange("(i p) t -> p i t", i=NC))
                sq = sb.tile([128, DK, TB], BF16, name="sqep", tag="sqep", bufs=1)
                nc.vector.tensor_mul(sq[:], x[:], x[:])
                stats = ps.tile([2, TB], F32, name="stats", tag="pqkv", bufs=2)
                for i in range(DK):
                    nc.tensor.matmul(stats[0:1, :], ones128[:, 0:1], x[:, i, :],
                                     start=(i == 0), stop=(i == DK - 1))
                for i in range(DK):
                    nc.tensor.matmul(stats[1:2, :], ones128[:, 0:1], sq[:, i, :],
                                     start=(i == 0), stop=(i == DK - 1),
                                     tile_position=(0, 0))
                # mq2 row0 = -mu, row1 = 1/rstd; rstd = exp(-0.5*ln(var+eps))
                mq2 = sb.tile([2, TB], BF16, name="mq2", tag="mq2", bufs=2)
                nc.vector.tensor_scalar_mul(mq2[0:1, :], stats[0:1, :], -1.0 / D)
                musq = sb.tile([1, TB], F32, name="musq", tag="musq", bufs=2)
                nc.vector.tensor_mul(musq[:], mq2[0:1, :], mq2[0:1, :])
                var = sb.tile([1, TB], F32, name="var", tag="var", bufs=2)
                nc.vector.scalar_tensor_tensor(var[:], stats[1:2, :], 1.0 / D,
                                               musq[:], op0=ALU.mult,
                                               op1=ALU.subtract)
                lnv = sb.tile([1, TB], F32, name="lnv", tag="lnv", bufs=2)
                nc.scalar.activation(lnv[:], var[:], AF.Ln, bias=epsc[:, 0:1])
                rstd = sb.tile([1, TB], F32, name="rstd", tag="rstd", bufs=2)
                nc.scalar.activation(rstd[:], lnv[:], AF.Exp, scale=-0.5)
                nc.scalar.activation(mq2[1:2, :], lnv[:], AF.Exp, scale=0.5)
                rb = sb.tile([128, TB], F32, name="rb", tag="rb", bufs=2)
                nc.gpsimd.partition_broadcast(rb[:], rstd[:])
                for occ in range(DK):
                    osl = slice(occ * 128, (occ + 1) * 128)
                    pp = ps.tile([128, TB], F32, name="pp", tag="pqkv", bufs=2)
                    for k in range(DK):
                        nc.tensor.matmul(pp[:], wpt[:, k, osl], x[:, k, :],
                                         start=(k == 0), stop=False)
                    nc.tensor.matmul(pp[:], wb2[:, occ, :], mq2[:],
                                     start=False, stop=True)
                    ou = sb.tile([128, TB], F32, name="ou", tag="ou", bufs=3)
                    nc.vector.tensor_mul(ou[:], pp[:], rb[:])
                    nc.sync.dma_start(out[occ, :, b * TB:(b + 1) * TB], ou[:])

            # ================= schedule =================
            # batch 0 QKV: K/V modality b first, then Qa so pass 0 starts early
            for ch in range(4):
                emit_qkv_chunk("b", ch)
            emit_qkv_chunk("a", 0)
            nc.sync.dma_start(wpt[:], wpt_in[:])
            for b in range(B):
                fused_sb[b] = sb.tile([128, N], BF16, name=f"fsb{b}",
                                      tag="fused", bufs=2)
                # feeds: remaining own-batch chunks (b=0) + next-batch chunks,
                # distributed round-robin over this batch's 8 (p, qc) slots
                feeds = [("a", i) for i in (1, 2, 3)] if b == 0 else []
                if b < B - 1:
                    feeds += [("b", 4 * (b + 1) + i) for i in range(4)]
                    feeds += [("a", 4 * (b + 1) + i) for i in range(4)]
                slots = [[] for _ in range(8)]
                for i, f in enumerate(feeds):
                    slots[i % 8].append(f)
                for p in range(2):
                    for qc in range(QC):
                        emit_attn_qc(b, p, qc, qkv_feeds=slots[p * 4 + qc])
                emit_a2a(b)
                emit_epilogue(b)

    nc.compile()
    return nc


_NC_CACHE = None


def _get_nc():
    global _NC_CACHE
    if _NC_CACHE is None:
        _NC_CACHE = _build_nc()
    return _NC_CACHE


def _prep_w(w, dtype=ml_dtypes.bfloat16):
    """[out_rows, D] -> transposed chunked [128, DK, out_rows]."""
    wt = np.ascontiguousarray(np.asarray(w, np.float32).T)  # [D, out]
    o = wt.shape[1]
    return np.ascontiguousarray(
        wt.reshape(DK, 128, o).transpose(1, 0, 2)).astype(dtype)


def _make_in_maps(inputs):
    f_a = np.asarray(inputs["f_a"], np.float32).reshape(T, D)
    f_b = np.asarray(inputs["f_b"], np.float32).reshape(T, D)
    gamma = np.asarray(inputs["ln_gamma"], np.float32)
    beta = np.asarray(inputs["ln_beta"], np.float32)
    W_proj = np.asarray(inputs["W_proj"], np.float32)
    b_proj = np.asarray(inputs["b_proj"], np.float32)

    xt_a = _prep_w(f_a)  # [128, DK, T] bf16
    xt_b = _prep_w(f_b)

    # fold LN gamma/beta into projection: y = (ln01(x)*g+bt) @ Wp^T + bp
    w_eff = W_proj * gamma[None, :]
    b_eff = (b_proj + beta @ W_proj.T).astype(np.float32)
    wsum = w_eff.sum(axis=1).astype(np.float32)
    wpt = _prep_w(w_eff)  # [128, DK, D]
    # wb2[0, occ, :] = wsum chunk, wb2[1, occ, :] = beff chunk
    wb2 = np.stack([wsum.reshape(DK, 128), b_eff.reshape(DK, 128)],
                   axis=0).astype(ml_dtypes.bfloat16)

    wmap = {"wq": "W_q_", "wk": "W_k_", "wv": "W_v_"}
    bmap = {"bq": "b_q_", "bk": "b_k_", "bv": "b_v_"}

    in_maps = []
    for c in range(NC):
        sl = slice(c * S, (c + 1) * S)
        m = {"xt_a": xt_a, "xt_b": xt_b, "wpt": wpt, "wb2": wb2,
             "onesrow": np.ones((1, 128), ml_dtypes.bfloat16),
             "ones128": np.ones((128, 1), ml_dtypes.bfloat16)}
        for mod in "ab":
            for dev, base in wmap.items():
                m[f"{dev}_{mod}"] = _prep_w(
                    np.asarray(inputs[base + mod], np.float32)[sl, :])
            for dev, base in bmap.items():
                bvals = np.asarray(inputs[base + mod], np.float32)[sl]
                if dev == "bv":
                    m[f"{dev}_{mod}"] = np.ascontiguousarray(
                        bvals.reshape(1, S)).astype(ml_dtypes.bfloat16)
                else:
                    m[f"{dev}_{mod}"] = np.ascontiguousarray(
                        bvals.reshape(S, 1))
        in_maps.append(m)
    return in_maps


def _assemble(outs):
    """outs: per-core [DK, 128, B*TB] f32 -> [B, N, D]."""
    full = np.empty((B, N, D), np.float32)
    for c in range(NC):
        oc = np.asarray(outs[c], np.float32).reshape(D, B, TB)
        full[:, c * TB:(c + 1) * TB, :] = oc.transpose(1, 2, 0)
    return full


def kernel(**inputs):
    in_maps = _make_in_maps(inputs)
    nc = _get_nc()
    res = run_bass_kernel_spmd(nc, in_maps, list(range(NC)))
    return _assemble([res.results[c]["out"] for c in range(NC)])
